# revision 11
# baseline (speedup 1.0000x reference)
"""LoRA Linear (T=8192, D_in=D_out=4096, r=16) on 8 TRN2 NeuronCores.

out = x @ W^T + b + (32/16) * ((x_bf16 @ A^T) @ B^T)

Strategy: data-parallel over the 8192-token axis (1024 tokens/core).
The rank-16 LoRA update is folded into the weight on the host:
  W_eff = W + 2.0 * (B @ A)   (fp32 accumulate)
so the device kernel is a single dense GEMM + bias.

The contraction (d=4096) is split into two precision regions:
  - k in [0, 256*N_DR): fp8(e4m3) operands with DoubleRow perf mode.
    DoubleRow packs 2 contraction indices per PE cell, so one matmul
    consumes 256 k at the same 216ns issue rate as a 128-k bf16 matmul
    (2x MAC throughput). Operands are pre-scaled (x*SX, W*SW) on the
    host so W leaves e4m3's subnormal range; the PSUM partial is scaled
    back by 1/(SX*SW) on the DVE combine.
  - remaining k: bf16 operands (full accuracy).
The fp8 region size N_DR is chosen so the deterministic end-to-end
rel-err (measured in an exact host sim) stays ~10% under the 2e-2 gate.

All SBUF stream tiles are padded to 2KB/partition: 1KB-strided tiles
measurably slow PE rhs streaming (259ns vs 216ns per matmul).
"""

import numpy as np

try:
    import concourse  # noqa: F401
except ImportError:  # pragma: no cover
    import sys

    sys.path.insert(0, "/opt/trn_rl_repo")

from concourse import bacc, mybir, tile
from concourse.bass_utils import run_bass_kernel_spmd

N_CORES = 8
T, D_IN, D_OUT, R = 8192, 4096, 4096, 16
TPC = T // N_CORES  # 1024 tokens per core
OC = 512  # output-column chunk (one PSUM bank of fp32)
N_OC = D_OUT // OC  # 8

N_DR = 9  # fp8-DoubleRow k-chunks (256 k each)
K8 = 256 * N_DR  # fp8 k-range
N_BF = (D_IN - K8) // 128  # bf16 k-chunks (128 k each)
SX, SW = 2.0, 32.0  # host pre-scales for fp8 operands
INV_S = 1.0 / (SX * SW)

N_TG = 2  # token groups per core (4 token tiles of 128 each)
TG = 4

f32 = mybir.dt.float32
bf16 = mybir.dt.bfloat16
f8e4 = mybir.dt.float8e4

_NC_CACHE = {}


def build_nc():
    nc = bacc.Bacc(
        "TRN2", target_bir_lowering=False, debug=False, num_devices=N_CORES
    )
    xT8 = nc.dram_tensor(
        "xT8", [N_DR * 128, 2 * TPC], f8e4, kind="ExternalInput"
    ).ap()
    xTb = nc.dram_tensor(
        "xTb", [N_BF * 128, TPC], bf16, kind="ExternalInput"
    ).ap()
    W8 = nc.dram_tensor(
        "W8", [N_OC * N_DR * 128, 2 * OC], f8e4, kind="ExternalInput"
    ).ap()
    Wbf = nc.dram_tensor(
        "Wbf", [N_OC * N_BF * 128, OC], bf16, kind="ExternalInput"
    ).ap()
    bias = nc.dram_tensor("bias", [128, D_OUT], f32, kind="ExternalInput").ap()
    out = nc.dram_tensor("out", [TPC, D_OUT], f32, kind="ExternalOutput").ap()

    with tile.TileContext(nc) as tc:
        with (
            tc.tile_pool(name="persist", bufs=1) as persist,
            tc.tile_pool(name="x8pool", bufs=N_DR) as x8pool,
            tc.tile_pool(name="xbpool", bufs=N_BF) as xbpool,
            tc.tile_pool(name="w8pool", bufs=6) as w8pool,
            tc.tile_pool(name="wbpool", bufs=8) as wbpool,
            tc.tile_pool(name="opool", bufs=8) as opool,
            tc.tile_pool(name="pspool", bufs=8, space="PSUM") as pspool,
        ):
            bias_sb = persist.tile([128, D_OUT], f32, tag="bias")
            xt8_tiles = [None] * N_DR
            xtb_tiles = [None] * N_BF

            for oc in range(N_OC):
                osl = slice(oc * OC, (oc + 1) * OC)
                for tg in range(N_TG):
                    first = oc == 0 and tg == 0
                    ps_f = [
                        pspool.tile(
                            [128, OC], f32, tag="ps", name=f"psf_{oc}_{tg}_{i}"
                        )
                        for i in range(TG)
                    ]
                    ps_b = [
                        pspool.tile(
                            [128, OC], f32, tag="ps", name=f"psb_{oc}_{tg}_{i}"
                        )
                        for i in range(TG)
                    ]
                    # fp8 DoubleRow region: k in [0, K8)
                    for dcp in range(N_DR):
                        if first:
                            xt8 = x8pool.tile([128, 2 * TPC], f8e4, tag="x8")
                            nc.sync.dma_start(
                                out=xt8[:],
                                in_=xT8[dcp * 128 : (dcp + 1) * 128, :],
                            )
                            xt8_tiles[dcp] = xt8
                        wt8 = w8pool.tile([128, 4 * OC], f8e4, tag="w8")
                        base = (oc * N_DR + dcp) * 128
                        nc.sync.dma_start(
                            out=wt8[:, 0 : 2 * OC],
                            in_=W8[base : base + 128, :],
                        )
                        rhs3 = wt8[:, 0 : 2 * OC].rearrange(
                            "p (a o) -> p a o", a=2
                        )
                        lhs3 = xt8_tiles[dcp][:].rearrange(
                            "p (a t) -> p a t", a=2
                        )
                        for ti in range(TG):
                            t = tg * TG + ti
                            nc.tensor.matmul(
                                ps_f[ti][:],
                                lhs3[:, :, t * 128 : (t + 1) * 128],
                                rhs3,
                                start=(dcp == 0),
                                stop=(dcp == N_DR - 1),
                                perf_mode=mybir.MatmulPerfMode.DoubleRow,
                            )
                    # bf16 region: k in [K8, D_IN)
                    for dcb in range(N_BF):
                        if first:
                            xtb = xbpool.tile([128, TPC], bf16, tag="xb")
                            nc.sync.dma_start(
                                out=xtb[:],
                                in_=xTb[dcb * 128 : (dcb + 1) * 128, :],
                            )
                            xtb_tiles[dcb] = xtb
                        wtb = wbpool.tile([128, 2 * OC], bf16, tag="wb")
                        base = (oc * N_BF + dcb) * 128
                        nc.sync.dma_start(
                            out=wtb[:, 0:OC], in_=Wbf[base : base + 128, :]
                        )
                        for ti in range(TG):
                            t = tg * TG + ti
                            nc.tensor.matmul(
                                ps_b[ti][:],
                                xtb_tiles[dcb][:, t * 128 : (t + 1) * 128],
                                wtb[:, 0:OC],
                                start=(dcb == 0),
                                stop=(dcb == N_BF - 1),
                            )
                        if first and dcb == 4:
                            nc.sync.dma_start(out=bias_sb[:], in_=bias[:])
                    # combine: out = psF/(SX*SW) + psB + bias
                    for ti in range(TG):
                        t = tg * TG + ti
                        o1 = opool.tile([128, OC], f32, tag="o1")
                        nc.vector.tensor_scalar_mul(
                            o1[:], ps_f[ti][:], INV_S
                        )
                        o2 = opool.tile([128, OC], f32, tag="o2")
                        nc.vector.tensor_tensor(
                            o2[:], o1[:], ps_b[ti][:], mybir.AluOpType.add
                        )
                        o3 = opool.tile([128, OC], f32, tag="o3")
                        nc.vector.tensor_tensor(
                            o3[:], o2[:], bias_sb[:, osl], mybir.AluOpType.add
                        )
                        nc.sync.dma_start(
                            out=out[t * 128 : (t + 1) * 128, osl], in_=o3[:]
                        )

    nc.compile()
    return nc


def _prepare_in_maps(x, W, b, lora_a, lora_b):
    import ml_dtypes

    E4 = ml_dtypes.float8_e4m3fn
    BF = ml_dtypes.bfloat16

    # Fold the LoRA update into the weight (fp32 math).
    W_eff = W + 2.0 * (
        lora_b.astype(np.float32) @ lora_a.astype(np.float32)
    )  # [D_OUT, D_IN]

    # fp8 region of W: [D_OUT, K8] -> blocked [(oc,dcp,p), (a,o)]
    Wq = np.clip(W_eff[:, :K8] * np.float32(SW), -240, 240).astype(E4)
    Wt8 = np.ascontiguousarray(
        Wq.T.reshape(N_DR, 2, 128, N_OC, OC)
        .transpose(3, 0, 2, 1, 4)
        .reshape(N_OC * N_DR * 128, 2 * OC)
    )
    # bf16 region of W: [D_OUT, K8:] -> blocked [(oc,dcb,p), o]
    Wb16 = W_eff[:, K8:].astype(BF)
    Wbf = np.ascontiguousarray(
        Wb16.T.reshape(N_BF, 128, N_OC, OC)
        .transpose(2, 0, 1, 3)
        .reshape(N_OC * N_BF * 128, OC)
    )
    bias = np.ascontiguousarray(
        np.broadcast_to(b.astype(np.float32), (128, D_OUT))
    )

    xq_full = np.clip(x[:, :K8] * np.float32(SX), -240, 240).astype(E4)
    xb_full = x[:, K8:].astype(BF)

    in_maps = []
    for c in range(N_CORES):
        tsl = slice(c * TPC, (c + 1) * TPC)
        xT8 = np.ascontiguousarray(
            xq_full[tsl].T.reshape(N_DR, 2, 128, TPC)
            .transpose(0, 2, 1, 3)
            .reshape(N_DR * 128, 2 * TPC)
        )
        xTb = np.ascontiguousarray(xb_full[tsl].T)
        in_maps.append(
            {"xT8": xT8, "xTb": xTb, "W8": Wt8, "Wbf": Wbf, "bias": bias}
        )
    return in_maps


def run(inputs, trace=False, **trace_kwargs):
    """Run on hardware; returns (full_output, BassKernelResults)."""
    if "nc" not in _NC_CACHE:
        _NC_CACHE["nc"] = build_nc()
    nc = _NC_CACHE["nc"]
    in_maps = _prepare_in_maps(
        np.asarray(inputs["x"], dtype=np.float32),
        np.asarray(inputs["W"], dtype=np.float32),
        np.asarray(inputs["b"], dtype=np.float32),
        np.asarray(inputs["lora_a"]),
        np.asarray(inputs["lora_b"]),
    )
    res = run_bass_kernel_spmd(
        nc, in_maps, list(range(N_CORES)), trace=trace, **trace_kwargs
    )
    out = np.concatenate(
        [res.results[c]["out"] for c in range(N_CORES)], axis=0
    )
    return out.astype(np.float32), res


def kernel(**inputs):
    out, _ = run(inputs, trace=False)
    return out


if __name__ == "__main__":
    rng = np.random.default_rng(0)
    import ml_dtypes

    x = rng.standard_normal((T, D_IN), dtype=np.float32)
    W = rng.standard_normal((D_OUT, D_IN), dtype=np.float32) * 0.02
    b = rng.standard_normal((D_OUT,), dtype=np.float32) * 0.02
    la = (rng.standard_normal((R, D_IN), dtype=np.float32) * 0.02).astype(
        ml_dtypes.bfloat16
    )
    lb = (rng.standard_normal((D_OUT, R), dtype=np.float32) * 0.02).astype(
        ml_dtypes.bfloat16
    )
    got = kernel(x=x, W=W, b=b, lora_a=la, lora_b=lb)
    ref = (
        x @ W.T
        + b
        + 2.0
        * (
            (x.astype(ml_dtypes.bfloat16).astype(np.float32) @ la.astype(np.float32).T)
            @ lb.astype(np.float32).T
        )
    )
    err = np.abs(got - ref).max() / np.abs(ref).max()
    print("scale-relative max err:", err)


# revision 13
# speedup vs baseline: 1.1760x; 1.1760x over previous
"""LoRA Linear (T=8192, D_in=D_out=4096, r=16) on 8 TRN2 NeuronCores.

out = x @ W^T + b + (32/16) * ((x_bf16 @ A^T) @ B^T)

Strategy: data-parallel over the 8192-token axis (1024 tokens/core).
The rank-16 LoRA update is folded into the weight on the host:
  W_eff = W + 2.0 * (B @ A)   (fp32 accumulate)
so the device kernel is a single dense GEMM + bias.

The contraction (d=4096) is split into two precision regions:
  - k in [0, 256*N_DR): fp8(e4m3) operands with DoubleRow perf mode.
    DoubleRow packs 2 contraction indices per PE cell, so one matmul
    consumes 256 k at the same 216ns issue rate as a 128-k bf16 matmul
    (2x MAC throughput). Operands are pre-scaled (x*SX, W*SW) on the
    host so W leaves e4m3's subnormal range; the PSUM partial is scaled
    back by 1/(SX*SW) on the DVE combine.
  - remaining k: bf16 operands (full accuracy).
The fp8 region size N_DR is chosen so the deterministic end-to-end
rel-err (measured in an exact host sim) stays ~10% under the 2e-2 gate.

All SBUF stream tiles are padded to 2KB/partition: 1KB-strided tiles
measurably slow PE rhs streaming (259ns vs 216ns per matmul).
"""

import numpy as np

try:
    import concourse  # noqa: F401
except ImportError:  # pragma: no cover
    import sys

    sys.path.insert(0, "/opt/trn_rl_repo")

from concourse import bacc, mybir, tile
from concourse.bass_utils import run_bass_kernel_spmd

N_CORES = 8
T, D_IN, D_OUT, R = 8192, 4096, 4096, 16
TPC = T // N_CORES  # 1024 tokens per core
OC = 512  # output-column chunk (one PSUM bank of fp32)
N_OC = D_OUT // OC  # 8

N_DR = 9  # fp8-DoubleRow k-chunks (256 k each)
K8 = 256 * N_DR  # fp8 k-range
N_BF = (D_IN - K8) // 128  # bf16 k-chunks (128 k each)
SX, SW = 2.0, 32.0  # host pre-scales for fp8 operands
INV_S = 1.0 / (SX * SW)

N_TG = 2  # token groups per core (4 token tiles of 128 each)
TG = 4

f32 = mybir.dt.float32
bf16 = mybir.dt.bfloat16
f8e4 = mybir.dt.float8e4

_NC_CACHE = {}


def build_nc():
    nc = bacc.Bacc(
        "TRN2", target_bir_lowering=False, debug=False, num_devices=N_CORES
    )
    xT8 = nc.dram_tensor(
        "xT8", [N_DR * 128, 2 * TPC], f8e4, kind="ExternalInput"
    ).ap()
    xTb = nc.dram_tensor(
        "xTb", [N_BF * 128, TPC], bf16, kind="ExternalInput"
    ).ap()
    W8 = nc.dram_tensor(
        "W8", [N_OC * N_DR * 128, 2 * OC], f8e4, kind="ExternalInput"
    ).ap()
    Wbf = nc.dram_tensor(
        "Wbf", [N_OC * N_BF * 128, OC], bf16, kind="ExternalInput"
    ).ap()
    bias = nc.dram_tensor("bias", [128, D_OUT], f32, kind="ExternalInput").ap()
    out = nc.dram_tensor("out", [TPC, D_OUT], f32, kind="ExternalOutput").ap()

    with tile.TileContext(nc) as tc:
        with (
            tc.tile_pool(name="persist", bufs=1) as persist,
            tc.tile_pool(name="x8pool", bufs=N_DR) as x8pool,
            tc.tile_pool(name="xbpool", bufs=N_BF) as xbpool,
            tc.tile_pool(name="w8pool", bufs=6) as w8pool,
            tc.tile_pool(name="wbpool", bufs=8) as wbpool,
            tc.tile_pool(name="opool", bufs=8) as opool,
            tc.tile_pool(name="pspool", bufs=8, space="PSUM") as pspool,
        ):
            bias_sb = persist.tile([128, D_OUT], f32, tag="bias")
            # 2KB/partition spacer: keeps the W stream pools at the same
            # 4KB-aligned SBUF bases as the N_DR=8 layout (odd x8pool size
            # shifts them by 2KB, which costs 259ns vs 216ns per matmul).
            _pad = persist.tile(
                [128, 2048], mybir.dt.uint8, tag="pad", name="pad"
            )
            xt8_tiles = [None] * N_DR
            xtb_tiles = [None] * N_BF

            for oc in range(N_OC):
                osl = slice(oc * OC, (oc + 1) * OC)
                for tg in range(N_TG):
                    first = oc == 0 and tg == 0
                    ps_f = [
                        pspool.tile(
                            [128, OC], f32, tag="ps", name=f"psf_{oc}_{tg}_{i}"
                        )
                        for i in range(TG)
                    ]
                    ps_b = [
                        pspool.tile(
                            [128, OC], f32, tag="ps", name=f"psb_{oc}_{tg}_{i}"
                        )
                        for i in range(TG)
                    ]
                    # fp8 DoubleRow region: k in [0, K8)
                    for dcp in range(N_DR):
                        if first:
                            xt8 = x8pool.tile([128, 2 * TPC], f8e4, tag="x8")
                            nc.sync.dma_start(
                                out=xt8[:],
                                in_=xT8[dcp * 128 : (dcp + 1) * 128, :],
                            )
                            xt8_tiles[dcp] = xt8
                        wt8 = w8pool.tile([128, 4 * OC], f8e4, tag="w8")
                        base = (oc * N_DR + dcp) * 128
                        nc.sync.dma_start(
                            out=wt8[:, 0 : 2 * OC],
                            in_=W8[base : base + 128, :],
                        )
                        rhs3 = wt8[:, 0 : 2 * OC].rearrange(
                            "p (a o) -> p a o", a=2
                        )
                        lhs3 = xt8_tiles[dcp][:].rearrange(
                            "p (a t) -> p a t", a=2
                        )
                        for ti in range(TG):
                            t = tg * TG + ti
                            nc.tensor.matmul(
                                ps_f[ti][:],
                                lhs3[:, :, t * 128 : (t + 1) * 128],
                                rhs3,
                                start=(dcp == 0),
                                stop=(dcp == N_DR - 1),
                                perf_mode=mybir.MatmulPerfMode.DoubleRow,
                            )
                    # bf16 region: k in [K8, D_IN)
                    for dcb in range(N_BF):
                        if first:
                            xtb = xbpool.tile([128, TPC], bf16, tag="xb")
                            nc.sync.dma_start(
                                out=xtb[:],
                                in_=xTb[dcb * 128 : (dcb + 1) * 128, :],
                            )
                            xtb_tiles[dcb] = xtb
                        wtb = wbpool.tile([128, 2 * OC], bf16, tag="wb")
                        base = (oc * N_BF + dcb) * 128
                        nc.sync.dma_start(
                            out=wtb[:, 0:OC], in_=Wbf[base : base + 128, :]
                        )
                        for ti in range(TG):
                            t = tg * TG + ti
                            nc.tensor.matmul(
                                ps_b[ti][:],
                                xtb_tiles[dcb][:, t * 128 : (t + 1) * 128],
                                wtb[:, 0:OC],
                                start=(dcb == 0),
                                stop=(dcb == N_BF - 1),
                            )
                        if first and dcb == 4:
                            nc.sync.dma_start(out=bias_sb[:], in_=bias[:])
                    # combine: out = psF/(SX*SW) + psB + bias
                    for ti in range(TG):
                        t = tg * TG + ti
                        o1 = opool.tile([128, OC], f32, tag="o1")
                        nc.vector.tensor_scalar_mul(
                            o1[:], ps_f[ti][:], INV_S
                        )
                        o2 = opool.tile([128, OC], f32, tag="o2")
                        nc.vector.tensor_tensor(
                            o2[:], o1[:], ps_b[ti][:], mybir.AluOpType.add
                        )
                        o3 = opool.tile([128, OC], f32, tag="o3")
                        nc.vector.tensor_tensor(
                            o3[:], o2[:], bias_sb[:, osl], mybir.AluOpType.add
                        )
                        nc.sync.dma_start(
                            out=out[t * 128 : (t + 1) * 128, osl], in_=o3[:]
                        )

    nc.compile()
    return nc


def _prepare_in_maps(x, W, b, lora_a, lora_b):
    import ml_dtypes

    E4 = ml_dtypes.float8_e4m3fn
    BF = ml_dtypes.bfloat16

    # Fold the LoRA update into the weight (fp32 math).
    W_eff = W + 2.0 * (
        lora_b.astype(np.float32) @ lora_a.astype(np.float32)
    )  # [D_OUT, D_IN]

    # fp8 region of W: [D_OUT, K8] -> blocked [(oc,dcp,p), (a,o)]
    Wq = np.clip(W_eff[:, :K8] * np.float32(SW), -240, 240).astype(E4)
    Wt8 = np.ascontiguousarray(
        Wq.T.reshape(N_DR, 2, 128, N_OC, OC)
        .transpose(3, 0, 2, 1, 4)
        .reshape(N_OC * N_DR * 128, 2 * OC)
    )
    # bf16 region of W: [D_OUT, K8:] -> blocked [(oc,dcb,p), o]
    Wb16 = W_eff[:, K8:].astype(BF)
    Wbf = np.ascontiguousarray(
        Wb16.T.reshape(N_BF, 128, N_OC, OC)
        .transpose(2, 0, 1, 3)
        .reshape(N_OC * N_BF * 128, OC)
    )
    bias = np.ascontiguousarray(
        np.broadcast_to(b.astype(np.float32), (128, D_OUT))
    )

    xq_full = np.clip(x[:, :K8] * np.float32(SX), -240, 240).astype(E4)
    xb_full = x[:, K8:].astype(BF)

    in_maps = []
    for c in range(N_CORES):
        tsl = slice(c * TPC, (c + 1) * TPC)
        xT8 = np.ascontiguousarray(
            xq_full[tsl].T.reshape(N_DR, 2, 128, TPC)
            .transpose(0, 2, 1, 3)
            .reshape(N_DR * 128, 2 * TPC)
        )
        xTb = np.ascontiguousarray(xb_full[tsl].T)
        in_maps.append(
            {"xT8": xT8, "xTb": xTb, "W8": Wt8, "Wbf": Wbf, "bias": bias}
        )
    return in_maps


def run(inputs, trace=False, **trace_kwargs):
    """Run on hardware; returns (full_output, BassKernelResults)."""
    if "nc" not in _NC_CACHE:
        _NC_CACHE["nc"] = build_nc()
    nc = _NC_CACHE["nc"]
    in_maps = _prepare_in_maps(
        np.asarray(inputs["x"], dtype=np.float32),
        np.asarray(inputs["W"], dtype=np.float32),
        np.asarray(inputs["b"], dtype=np.float32),
        np.asarray(inputs["lora_a"]),
        np.asarray(inputs["lora_b"]),
    )
    res = run_bass_kernel_spmd(
        nc, in_maps, list(range(N_CORES)), trace=trace, **trace_kwargs
    )
    out = np.concatenate(
        [res.results[c]["out"] for c in range(N_CORES)], axis=0
    )
    return out.astype(np.float32), res


def kernel(**inputs):
    out, _ = run(inputs, trace=False)
    return out


if __name__ == "__main__":
    rng = np.random.default_rng(0)
    import ml_dtypes

    x = rng.standard_normal((T, D_IN), dtype=np.float32)
    W = rng.standard_normal((D_OUT, D_IN), dtype=np.float32) * 0.02
    b = rng.standard_normal((D_OUT,), dtype=np.float32) * 0.02
    la = (rng.standard_normal((R, D_IN), dtype=np.float32) * 0.02).astype(
        ml_dtypes.bfloat16
    )
    lb = (rng.standard_normal((D_OUT, R), dtype=np.float32) * 0.02).astype(
        ml_dtypes.bfloat16
    )
    got = kernel(x=x, W=W, b=b, lora_a=la, lora_b=lb)
    ref = (
        x @ W.T
        + b
        + 2.0
        * (
            (x.astype(ml_dtypes.bfloat16).astype(np.float32) @ la.astype(np.float32).T)
            @ lb.astype(np.float32).T
        )
    )
    err = np.abs(got - ref).max() / np.abs(ref).max()
    print("scale-relative max err:", err)


# revision 15
# speedup vs baseline: 1.1780x; 1.0017x over previous
"""LoRA Linear (T=8192, D_in=D_out=4096, r=16) on 8 TRN2 NeuronCores.

out = x @ W^T + b + (32/16) * ((x_bf16 @ A^T) @ B^T)

Strategy: data-parallel over the 8192-token axis (1024 tokens/core).
The rank-16 LoRA update is folded into the weight on the host:
  W_eff = W + 2.0 * (B @ A)   (fp32 accumulate)
so the device kernel is a single dense GEMM + bias.

The contraction (d=4096) is split into two precision regions:
  - k in [0, 256*N_DR): fp8(e4m3) operands with DoubleRow perf mode.
    DoubleRow packs 2 contraction indices per PE cell, so one matmul
    consumes 256 k at the same 216ns issue rate as a 128-k bf16 matmul
    (2x MAC throughput). Operands are pre-scaled (x*SX, W*SW) on the
    host so W leaves e4m3's subnormal range; the PSUM partial is scaled
    back by 1/(SX*SW) on the DVE combine.
  - remaining k: bf16 operands (full accuracy).
The fp8 region size N_DR is chosen so the deterministic end-to-end
rel-err (measured in an exact host sim) stays ~10% under the 2e-2 gate.

All SBUF stream tiles are padded to 2KB/partition: 1KB-strided tiles
measurably slow PE rhs streaming (259ns vs 216ns per matmul).
"""

import numpy as np

try:
    import concourse  # noqa: F401
except ImportError:  # pragma: no cover
    import sys

    sys.path.insert(0, "/opt/trn_rl_repo")

from concourse import bacc, mybir, tile
from concourse.bass_utils import run_bass_kernel_spmd

N_CORES = 8
T, D_IN, D_OUT, R = 8192, 4096, 4096, 16
TPC = T // N_CORES  # 1024 tokens per core
OC = 512  # output-column chunk (one PSUM bank of fp32)
N_OC = D_OUT // OC  # 8

N_DR = 9  # fp8-DoubleRow k-chunks (256 k each)
K8 = 256 * N_DR  # fp8 k-range
N_BF = (D_IN - K8) // 128  # bf16 k-chunks (128 k each)
SX, SW = 2.0, 32.0  # host pre-scales for fp8 operands
INV_S = 1.0 / (SX * SW)

N_TG = 2  # token groups per core (4 token tiles of 128 each)
TG = 4

f32 = mybir.dt.float32
bf16 = mybir.dt.bfloat16
f8e4 = mybir.dt.float8e4

_NC_CACHE = {}


def build_nc():
    nc = bacc.Bacc(
        "TRN2", target_bir_lowering=False, debug=False, num_devices=N_CORES
    )
    xT8 = nc.dram_tensor(
        "xT8", [N_DR * 128, 2 * TPC], f8e4, kind="ExternalInput"
    ).ap()
    xTb = nc.dram_tensor(
        "xTb", [N_BF * 128, TPC], bf16, kind="ExternalInput"
    ).ap()
    W8 = nc.dram_tensor(
        "W8", [N_OC * N_DR * 128, 2 * OC], f8e4, kind="ExternalInput"
    ).ap()
    Wbf = nc.dram_tensor(
        "Wbf", [N_OC * N_BF * 128, OC], bf16, kind="ExternalInput"
    ).ap()
    bias = nc.dram_tensor("bias", [128, D_OUT], f32, kind="ExternalInput").ap()
    out = nc.dram_tensor("out", [TPC, D_OUT], f32, kind="ExternalOutput").ap()

    with tile.TileContext(nc) as tc:
        with (
            tc.tile_pool(name="persist", bufs=1) as persist,
            tc.tile_pool(name="x8pool", bufs=N_DR) as x8pool,
            tc.tile_pool(name="xbpool", bufs=N_BF) as xbpool,
            tc.tile_pool(name="w8pool", bufs=6) as w8pool,
            tc.tile_pool(name="wbpool", bufs=8) as wbpool,
            tc.tile_pool(name="opool", bufs=8) as opool,
            tc.tile_pool(name="pspool", bufs=8, space="PSUM") as pspool,
        ):
            bias_sb = persist.tile([128, D_OUT], f32, tag="bias")
            # 2KB/partition spacer: keeps the W stream pools at the same
            # 4KB-aligned SBUF bases as the N_DR=8 layout (odd x8pool size
            # shifts them by 2KB, which costs 259ns vs 216ns per matmul).
            _pad = persist.tile(
                [128, 2048], mybir.dt.uint8, tag="pad", name="pad"
            )
            xt8_tiles = [None] * N_DR
            xtb_tiles = [None] * N_BF

            for oc in range(N_OC):
                osl = slice(oc * OC, (oc + 1) * OC)
                for tg in range(N_TG):
                    first = oc == 0 and tg == 0
                    ps_f = [
                        pspool.tile(
                            [128, OC], f32, tag="ps", name=f"psf_{oc}_{tg}_{i}"
                        )
                        for i in range(TG)
                    ]
                    ps_b = [
                        pspool.tile(
                            [128, OC], f32, tag="ps", name=f"psb_{oc}_{tg}_{i}"
                        )
                        for i in range(TG)
                    ]
                    # fp8 DoubleRow region: k in [0, K8)
                    for dcp in range(N_DR):
                        if first:
                            xt8 = x8pool.tile([128, 2 * TPC], f8e4, tag="x8")
                            nc.sync.dma_start(
                                out=xt8[:],
                                in_=xT8[dcp * 128 : (dcp + 1) * 128, :],
                            )
                            xt8_tiles[dcp] = xt8
                        wt8 = w8pool.tile([128, 4 * OC], f8e4, tag="w8")
                        base = (oc * N_DR + dcp) * 128
                        nc.sync.dma_start(
                            out=wt8[:, 0 : 2 * OC],
                            in_=W8[base : base + 128, :],
                        )
                        rhs3 = wt8[:, 0 : 2 * OC].rearrange(
                            "p (a o) -> p a o", a=2
                        )
                        lhs3 = xt8_tiles[dcp][:].rearrange(
                            "p (a t) -> p a t", a=2
                        )
                        for ti in range(TG):
                            t = tg * TG + ti
                            nc.tensor.matmul(
                                ps_f[ti][:],
                                lhs3[:, :, t * 128 : (t + 1) * 128],
                                rhs3,
                                start=(dcp == 0),
                                stop=(dcp == N_DR - 1),
                                perf_mode=mybir.MatmulPerfMode.DoubleRow,
                            )
                        if first:
                            # Prefetch bf16 x tiles behind this dcp's W tile
                            # so the bf16 section doesn't stall on them.
                            for dcb in range(2 * dcp, min(2 * dcp + 2, N_BF)):
                                xtb = xbpool.tile([128, TPC], bf16, tag="xb")
                                nc.sync.dma_start(
                                    out=xtb[:],
                                    in_=xTb[dcb * 128 : (dcb + 1) * 128, :],
                                )
                                xtb_tiles[dcb] = xtb
                    # bf16 region: k in [K8, D_IN)
                    for dcb in range(N_BF):
                        wtb = wbpool.tile([128, 2 * OC], bf16, tag="wb")
                        base = (oc * N_BF + dcb) * 128
                        nc.sync.dma_start(
                            out=wtb[:, 0:OC], in_=Wbf[base : base + 128, :]
                        )
                        for ti in range(TG):
                            t = tg * TG + ti
                            nc.tensor.matmul(
                                ps_b[ti][:],
                                xtb_tiles[dcb][:, t * 128 : (t + 1) * 128],
                                wtb[:, 0:OC],
                                start=(dcb == 0),
                                stop=(dcb == N_BF - 1),
                            )
                        if first and dcb == 4:
                            nc.sync.dma_start(out=bias_sb[:], in_=bias[:])
                    # combine: out = psF/(SX*SW) + psB + bias
                    for ti in range(TG):
                        t = tg * TG + ti
                        o1 = opool.tile([128, OC], f32, tag="o1")
                        nc.vector.tensor_scalar_mul(
                            o1[:], ps_f[ti][:], INV_S
                        )
                        o2 = opool.tile([128, OC], f32, tag="o2")
                        nc.vector.tensor_tensor(
                            o2[:], o1[:], ps_b[ti][:], mybir.AluOpType.add
                        )
                        o3 = opool.tile([128, OC], f32, tag="o3")
                        nc.vector.tensor_tensor(
                            o3[:], o2[:], bias_sb[:, osl], mybir.AluOpType.add
                        )
                        nc.sync.dma_start(
                            out=out[t * 128 : (t + 1) * 128, osl], in_=o3[:]
                        )

    nc.compile()
    return nc


def _prepare_in_maps(x, W, b, lora_a, lora_b):
    import ml_dtypes

    E4 = ml_dtypes.float8_e4m3fn
    BF = ml_dtypes.bfloat16

    # Fold the LoRA update into the weight (fp32 math).
    W_eff = W + 2.0 * (
        lora_b.astype(np.float32) @ lora_a.astype(np.float32)
    )  # [D_OUT, D_IN]

    # fp8 region of W: [D_OUT, K8] -> blocked [(oc,dcp,p), (a,o)]
    Wq = np.clip(W_eff[:, :K8] * np.float32(SW), -240, 240).astype(E4)
    Wt8 = np.ascontiguousarray(
        Wq.T.reshape(N_DR, 2, 128, N_OC, OC)
        .transpose(3, 0, 2, 1, 4)
        .reshape(N_OC * N_DR * 128, 2 * OC)
    )
    # bf16 region of W: [D_OUT, K8:] -> blocked [(oc,dcb,p), o]
    Wb16 = W_eff[:, K8:].astype(BF)
    Wbf = np.ascontiguousarray(
        Wb16.T.reshape(N_BF, 128, N_OC, OC)
        .transpose(2, 0, 1, 3)
        .reshape(N_OC * N_BF * 128, OC)
    )
    bias = np.ascontiguousarray(
        np.broadcast_to(b.astype(np.float32), (128, D_OUT))
    )

    xq_full = np.clip(x[:, :K8] * np.float32(SX), -240, 240).astype(E4)
    xb_full = x[:, K8:].astype(BF)

    in_maps = []
    for c in range(N_CORES):
        tsl = slice(c * TPC, (c + 1) * TPC)
        xT8 = np.ascontiguousarray(
            xq_full[tsl].T.reshape(N_DR, 2, 128, TPC)
            .transpose(0, 2, 1, 3)
            .reshape(N_DR * 128, 2 * TPC)
        )
        xTb = np.ascontiguousarray(xb_full[tsl].T)
        in_maps.append(
            {"xT8": xT8, "xTb": xTb, "W8": Wt8, "Wbf": Wbf, "bias": bias}
        )
    return in_maps


def run(inputs, trace=False, **trace_kwargs):
    """Run on hardware; returns (full_output, BassKernelResults)."""
    if "nc" not in _NC_CACHE:
        _NC_CACHE["nc"] = build_nc()
    nc = _NC_CACHE["nc"]
    in_maps = _prepare_in_maps(
        np.asarray(inputs["x"], dtype=np.float32),
        np.asarray(inputs["W"], dtype=np.float32),
        np.asarray(inputs["b"], dtype=np.float32),
        np.asarray(inputs["lora_a"]),
        np.asarray(inputs["lora_b"]),
    )
    res = run_bass_kernel_spmd(
        nc, in_maps, list(range(N_CORES)), trace=trace, **trace_kwargs
    )
    out = np.concatenate(
        [res.results[c]["out"] for c in range(N_CORES)], axis=0
    )
    return out.astype(np.float32), res


def kernel(**inputs):
    out, _ = run(inputs, trace=False)
    return out


if __name__ == "__main__":
    rng = np.random.default_rng(0)
    import ml_dtypes

    x = rng.standard_normal((T, D_IN), dtype=np.float32)
    W = rng.standard_normal((D_OUT, D_IN), dtype=np.float32) * 0.02
    b = rng.standard_normal((D_OUT,), dtype=np.float32) * 0.02
    la = (rng.standard_normal((R, D_IN), dtype=np.float32) * 0.02).astype(
        ml_dtypes.bfloat16
    )
    lb = (rng.standard_normal((D_OUT, R), dtype=np.float32) * 0.02).astype(
        ml_dtypes.bfloat16
    )
    got = kernel(x=x, W=W, b=b, lora_a=la, lora_b=lb)
    ref = (
        x @ W.T
        + b
        + 2.0
        * (
            (x.astype(ml_dtypes.bfloat16).astype(np.float32) @ la.astype(np.float32).T)
            @ lb.astype(np.float32).T
        )
    )
    err = np.abs(got - ref).max() / np.abs(ref).max()
    print("scale-relative max err:", err)
